# revision 1
# baseline (speedup 1.0000x reference)
"""DiT attention (B=2, T=2048, D=1024, H=16, rope on head 0) on 8 trn2 cores.

Sharding: tensor-parallel over heads. Core c owns heads {2c, 2c+1}:
  - QKV projection: column-sharded (384 features per core), x^T replicated.
  - Attention: fully local per (batch, head); computed transposed
    (S^T = K^T' @ Q^T per 128-key tile) so softmax's exp evicts PSUM->SBUF
    on the ACT engine; row-sums come free via an appended ones-column on V
    (out^T psum rows 0-63 = head out, row 64 = softmax denominator).
  - Out projection: row-sharded; per-core partial [4096, 1024] summed on host.
All matmuls run as float32r (full PE rate at N=512).
"""
import sys, os
sys.path.insert(0, "/opt/trn_rl_repo")
import numpy as np

B, T, D, H, HD = 2, 2048, 1024, 16, 64
NCORES = 8
NTOK = B * T            # 4096
TT = 4                  # token tiles of 512 per batch (projection)
KC = 8                  # contraction chunks of 128 over D
NKT = T // 128          # 16 key tiles
QC = 4                  # q chunks of 512 per batch
ROPE_BASE = 10000.0
REPEATS = 1  # >1: repeat the whole computation on-device (timing differential)

_CACHE = {}


def _build():
    import concourse.bacc as bacc
    import concourse.mybir as mybir
    import concourse.tile as tile

    F32 = mybir.dt.float32
    F32R = mybir.dt.float32r
    EXP = mybir.ActivationFunctionType.Exp

    nc = bacc.Bacc("TRN2", target_bir_lowering=False, debug=False, num_devices=NCORES)

    xT = nc.dram_tensor("xT", [D, NTOK], F32R, kind="ExternalInput")
    wqkv = nc.dram_tensor("wqkv", [D, 384], F32R, kind="ExternalInput")
    wout = nc.dram_tensor("wout", [128, D], F32R, kind="ExternalInput")
    cosT = nc.dram_tensor("cosT", [64, T], F32R, kind="ExternalInput")
    sinT = nc.dram_tensor("sinT", [64, T], F32R, kind="ExternalInput")
    maskb = nc.dram_tensor("maskb", [128, B * NKT], F32, kind="ExternalInput")
    ident = nc.dram_tensor("ident", [128, 128], F32, kind="ExternalInput")
    out_d = nc.dram_tensor("out", [NTOK, D], F32, kind="ExternalOutput")

    with tile.TileContext(nc) as tc:
        with (
            tc.tile_pool(name="consts", bufs=1) as consts,
            tc.tile_pool(name="resid", bufs=1) as resid,
            tc.tile_pool(name="vaugp", bufs=2) as vaugp,
            tc.tile_pool(name="xtp", bufs=16) as xtp,
            tc.tile_pool(name="ptp", bufs=5) as ptp,
            tc.tile_pool(name="outst", bufs=2) as outstp,
            tc.tile_pool(name="smallp", bufs=4) as smallp,
            tc.tile_pool(name="rotp", bufs=2) as rotp,
            tc.tile_pool(name="dramp", bufs=4, space="DRAM") as dramp,
            tc.tile_pool(name="ps_sc", bufs=2, space="PSUM") as ps_sc,
            tc.tile_pool(name="ps_av", bufs=1, space="PSUM") as ps_av,
            tc.tile_pool(name="ps_sm", bufs=2, space="PSUM") as ps_sm,
        ):
            # ---- constants ----
            wq_sb = []
            for kc in range(KC):
                wt = consts.tile([128, 384], F32R, name=f"wq{kc}")
                nc.sync.dma_start(wt[:], wqkv[kc * 128:(kc + 1) * 128, :])
                wq_sb.append(wt)
            wout_sb = consts.tile([128, D], F32R)
            nc.sync.dma_start(wout_sb[:], wout[:])
            cos_sb = consts.tile([64, T], F32R)
            nc.sync.dma_start(cos_sb[:], cosT[:])
            sin_sb = consts.tile([64, T], F32R)
            nc.sync.dma_start(sin_sb[:], sinT[:])
            mb_sb = consts.tile([128, B * NKT], F32)
            nc.sync.dma_start(mb_sb[:], maskb[:])
            id_sb = consts.tile([128, 128], F32)
            nc.sync.dma_start(id_sb[:], ident[:])

            # ---- resident per-batch tensors ----
            qt_sb = [resid.tile([128, T], F32R, name=f"qt{b}") for b in range(B)]
            kt_sb = [resid.tile([128, T], F32R, name=f"kt{b}") for b in range(B)]
            # V natural layout per (b, head): [k 128, 65] tiles (col 64 = ones)
            vnat = [[resid.tile([128, NKT * 65], F32R, name=f"vn{b}{h}") for h in range(2)]
                    for b in range(B)]

            def proj_gen(b):
                """QKV projection for batch b; yields between small work items
                so it can be interleaved into the previous batch's attention.
                Transposes + RoPE are pipelined per token-tile."""
                vaug = [vaugp.tile([65, T], F32, name=f"va{b}{h}", tag=f"vaug{h}")
                        for h in range(2)]
                for h in range(2):
                    nc.gpsimd.memset(vaug[h][64:65, :], 1.0)
                for tt in range(TT):
                    g0 = b * T + tt * 512
                    sl = slice(tt * 512, (tt + 1) * 512)
                    xts = []
                    for kc in range(KC):
                        xt_t = xtp.tile([128, 512], F32R, name=f"xt{b}{tt}{kc}", tag="xt")
                        # split each tile across two HWDGE queues
                        nc.sync.dma_start(xt_t[0:64, :], xT[kc * 128:kc * 128 + 64, g0:g0 + 512])
                        nc.sync.dma_start(xt_t[64:128, :], xT[kc * 128 + 64:(kc + 1) * 128, g0:g0 + 512])
                        xts.append(xt_t)
                    for ft in range(3):  # 0=Q, 1=K, 2=V
                        ps = ps_sm.tile([128, 512], F32, name=f"pp{b}{tt}{ft}", tag="sm")
                        for kc in range(KC):
                            nc.tensor.matmul(
                                ps[:], wq_sb[kc][:, ft * 128:(ft + 1) * 128], xts[kc][:],
                                start=(kc == 0), stop=(kc == KC - 1),
                            )
                        if ft == 0:
                            nc.vector.tensor_copy(qt_sb[b][:, sl], ps[:])
                        elif ft == 1:
                            nc.vector.tensor_copy(kt_sb[b][:, sl], ps[:])
                        else:
                            nc.vector.tensor_copy(vaug[0][0:64, sl], ps[0:64, :])
                            nc.vector.tensor_copy(vaug[1][0:64, sl], ps[64:128, :])
                        yield
                    # V_aug^T -> V_nat via PE transpose (this token-tile's 4 key tiles)
                    for h in range(2):
                        for kt in range(tt * 4, (tt + 1) * 4):
                            tp = ps_sm.tile([128, 65], F32, name=f"tp{b}{h}{kt}", tag="sm")
                            nc.tensor.transpose(tp[:], vaug[h][:, kt * 128:(kt + 1) * 128],
                                                id_sb[0:65, 0:65])
                            nc.vector.tensor_copy(vnat[b][h][:, kt * 65:(kt + 1) * 65], tp[:])
                            yield
                    # RoPE on head-even rows (0:64) of this token-tile's Q^T/K^T
                    # (identity data on cores != 0)
                    for t_sb in (qt_sb[b], kt_sb[b]):
                        rot = rotp.tile([64, 512], F32R, name=f"rot{b}{tt}", tag="rot")
                        nc.gpsimd.tensor_copy(rot[0:32, :], t_sb[32:64, sl])
                        nc.gpsimd.tensor_copy(rot[32:64, :], t_sb[0:32, sl])
                        yield
                        nc.vector.tensor_mul(rot[:], rot[:], sin_sb[:, sl])
                        nc.vector.tensor_mul(t_sb[0:64, sl], t_sb[0:64, sl], cos_sb[:, sl])
                        yield
                        nc.vector.tensor_add(t_sb[0:64, sl], t_sb[0:64, sl], rot[:])
                        yield

            def pull(bgs):
                while bgs:
                    try:
                        next(bgs[0])
                        return
                    except StopIteration:
                        bgs.pop(0)

            def attn_unit(b, qc, bgs):
                """Attention for (batch b, 512-query chunk qc); PE stream is
                software-pipelined (scores one key-tile ahead of AV).
                bgs: list of background generators to interleave (next batch's
                projection, previous chunk's normalize+out-proj tail).
                Returns this chunk's tail generator."""
                q0 = qc * 512
                av = [ps_av.tile([65, 512], F32, name=f"av{b}{qc}{h}", tag=f"av{h}")
                      for h in range(2)]
                sc_t = {}

                def trace_scores(kt):
                    sc = ps_sc.tile([128, 1024], F32, name=f"sc{b}{qc}{kt}", tag="sc")
                    for h in range(2):
                        nc.tensor.matmul(
                            sc[:, h * 512:(h + 1) * 512],
                            kt_sb[b][h * 64:(h + 1) * 64, kt * 128:(kt + 1) * 128],
                            qt_sb[b][h * 64:(h + 1) * 64, q0:q0 + 512],
                            start=True, stop=True,
                        )
                    sc_t[kt] = sc

                trace_scores(0)
                for kt in range(NKT):
                    if kt + 1 < NKT:
                        trace_scores(kt + 1)
                    pt = ptp.tile([128, 1024], F32R, name=f"pt{b}{qc}{kt}", tag="pt")
                    nc.scalar.activation(pt[:], sc_t.pop(kt)[:], EXP,
                                         bias=mb_sb[:, b * NKT + kt: b * NKT + kt + 1],
                                         scale=float(HD) ** -0.5)
                    for h in range(2):
                        nc.tensor.matmul(
                            av[h][:], vnat[b][h][:, kt * 65:(kt + 1) * 65],
                            pt[:, h * 512:(h + 1) * 512],
                            start=(kt == 0), stop=(kt == NKT - 1),
                        )
                    pull(bgs)

                # Evict AV psum to SBUF right away (frees the psum bank for the
                # next chunk's accumulation) and kick off the denominator
                # broadcast chain; the actual normalization is deferred.
                av_sb, bcasts = [], []
                for h in range(2):
                    srecip = smallp.tile([1, 512], F32, name=f"sr{b}{qc}{h}", tag="sr")
                    nc.vector.reciprocal(srecip[:], av[h][64:65, :])
                    avc = smallp.tile([64, 512], F32, name=f"avc{b}{qc}{h}", tag="avc")
                    nc.vector.tensor_copy(avc[:], av[h][0:64, :])
                    av_sb.append(avc)
                    srd = dramp.tile([1, 512], F32, name=f"srd{b}{qc}{h}", tag="srd")
                    nc.sync.dma_start(srd[:], srecip[:])
                    bcast = smallp.tile([64, 512], F32, name=f"bc{b}{qc}{h}", tag="bc")
                    nc.gpsimd.dma_start(bcast[:], srd[:].broadcast_to([64, 512]))
                    bcasts.append(bcast)

                def tail():
                    # normalize (deferred softmax division) -> stacked [128 d, 512 q]
                    out_st = outstp.tile([128, 512], F32R, name=f"os{b}{qc}", tag="os")
                    for h in range(2):
                        nc.vector.tensor_mul(out_st[h * 64:(h + 1) * 64, :],
                                             av_sb[h][:], bcasts[h][:])
                        yield
                    # out projection: partial = out_st.T @ wout_slice -> DRAM
                    for qt in range(4):
                        g = b * T + q0 + qt * 128
                        for nt in range(2):
                            po = ps_sm.tile([128, 512], F32, name=f"po{b}{qc}{qt}{nt}", tag="sm")
                            nc.tensor.matmul(
                                po[:], out_st[:, qt * 128:(qt + 1) * 128],
                                wout_sb[:, nt * 512:(nt + 1) * 512],
                                start=True, stop=True,
                            )
                            ob = smallp.tile([128, 512], F32, name=f"ob{b}{qc}{qt}{nt}", tag="ob")
                            nc.vector.tensor_copy(ob[:], po[:])
                            nc.sync.dma_start(out_d[g:g + 128, nt * 512:(nt + 1) * 512], ob[:])
                            yield

                return tail()

            # ---- schedule ----
            for _rep in range(REPEATS):
                for b in range(B):
                    for _ in proj_gen(b):
                        pass
                bgs = []
                for b in range(B):
                    for qc in range(QC):
                        t = attn_unit(b, qc, bgs)
                        bgs.append(t)
                for g in bgs:
                    for _ in g:
                        pass

    nc.compile()
    return nc


def _host_inputs(x, w_qkv, w_out, mask):
    x = np.asarray(x, dtype=np.float32)
    w_qkv = np.asarray(w_qkv, dtype=np.float32)
    w_out = np.asarray(w_out, dtype=np.float32)
    mask = np.asarray(mask)

    xT = np.ascontiguousarray(x.reshape(NTOK, D).T)

    inv_freq = 1.0 / (ROPE_BASE ** (np.arange(0, HD, 2, dtype=np.float32) / HD))
    t = np.arange(T, dtype=np.float32)
    freqs = np.outer(t, inv_freq)                    # [T, 32]
    cos_r = np.cos(np.concatenate([freqs, freqs], 1)).T.astype(np.float32)  # [64, T]
    sin_half = np.sin(freqs).T.astype(np.float32)    # [32, T]
    sin_r = np.concatenate([-sin_half, sin_half], 0)  # [64, T] signed

    mb = np.zeros((128, B * NKT), dtype=np.float32)
    for b in range(B):
        for kt in range(NKT):
            mb[:, b * NKT + kt] = np.where(mask[b, kt * 128:(kt + 1) * 128], 0.0, -1e30)

    ident = np.eye(128, dtype=np.float32)

    in_maps = []
    for c in range(NCORES):
        cs = slice(c * 128, (c + 1) * 128)
        wq_c = np.ascontiguousarray(np.concatenate(
            [w_qkv[:, 0:D][:, cs], w_qkv[:, D:2 * D][:, cs], w_qkv[:, 2 * D:3 * D][:, cs]], axis=1))
        if c == 0:
            cosc, sinc = cos_r, sin_r
        else:
            cosc = np.ones_like(cos_r)
            sinc = np.zeros_like(sin_r)
        in_maps.append({
            "xT": xT,
            "wqkv": wq_c,
            "wout": np.ascontiguousarray(w_out[cs, :]),
            "cosT": cosc,
            "sinT": sinc,
            "maskb": mb,
            "ident": ident,
        })
    return in_maps


def kernel(x, w_qkv, w_out, mask):
    if "nc" not in _CACHE:
        _CACHE["nc"] = _build()
    nc = _CACHE["nc"]
    in_maps = _host_inputs(x, w_qkv, w_out, mask)

    from concourse.bass_utils import run_bass_kernel_spmd
    res = run_bass_kernel_spmd(nc, in_maps, core_ids=list(range(NCORES)))
    _CACHE["last_results"] = res

    total = np.zeros((NTOK, D), dtype=np.float32)
    for c in range(NCORES):
        total += res.results[c]["out"]
    return total.reshape(B, T, D)

